# revision 2
# baseline (speedup 1.0000x reference)
"""Expert-parallel SwiGLU MLP (MoE experts) for 8 Trainium2 NeuronCores — v2.

Problem: routed_in_egD [E*G, D] fp32, w1/w3 [E, D, F], w2 [E, F, D], E=8,
G=2048, D=2048, F=5632.  reference:
    x_egD = routed.reshape(E, G, D)
    mid   = silu(x @ w1) * (x @ w3)          # [E, G, F]
    out   = (mid @ w2).reshape(E*G, D)

Sharding: expert-parallel — core e gets expert e's x slice + weights; no
collectives.  Each core runs three 2048x2048x5632-class GEMMs (~142 GFLOP).

v2 vs v1: all-bf16 matmul pipeline (same PE cycle count as fp32r at 512
moving, but half the SBUF footprint and no fp32r HW-rate risk); phase-2
re-reads of mid eliminated (92MB -> 23MB) by keeping a g-half of mid
SBUF-resident and accumulating fo-outer into 8 PSUM banks; w2 pre-cast to
bf16 in DRAM spread across phase 1 so phase-2 HBM demand drops 155->86MB;
split xT so phase-1 matmuls start after half the x transpose; first w2
panel prefetched during the phase-1 tail so the phase transition only
waits on the (chasing) mid panel load.

Per-core kernel:
  phase 0: x [G,D] fp32 --DMA-cast--> bf16, PE-transpose -> xT0/xT1
           [D, 1024] bf16 resident in SBUF.
  phase 1: per f-chunk (128 rows of F), per g-half: gateT/upT = w1/w3.T @ x
           accumulated over D in PSUM; SwiGLU (ACT silu -> bf16, DVE mul);
           midT [F,G] spilled to DRAM bf16 per g-half.  Side stream: w2
           fp32 -> bf16 recast via SBUF into DRAM scratch.
  phase 2: per g-half: midH [F,1024] bf16 SBUF-resident (fo-chunked load
           chases compute); per d-quarter: out[g,d] += midT.T @ w2, fo-outer
           over 8 PSUM banks (one per 128-g block).  dq order reversed on
           the second half so the last w2 panel is reused across halves.
"""

import numpy as np

import concourse.mybir as mybir
import concourse.tile as tile
from concourse import bacc
from concourse.bass_utils import run_bass_kernel_spmd
from concourse.masks import make_identity

E, G, D, F = 8, 2048, 2048, 5632
P = 128
DO = D // P      # 16 d-chunks
FC = F // P      # 44 f-chunks
GO = G // P      # 16 g-chunks
GH = G // 2      # 1024 g-half

F32 = mybir.dt.float32
BF16 = mybir.dt.bfloat16


def build_nc():
    nc = bacc.Bacc("TRN2", target_bir_lowering=False)
    x = nc.dram_tensor("x", [G, D], F32, kind="ExternalInput").ap()
    w1 = nc.dram_tensor("w1", [D, F], F32, kind="ExternalInput").ap()
    w2 = nc.dram_tensor("w2", [F, D], F32, kind="ExternalInput").ap()
    w3 = nc.dram_tensor("w3", [D, F], F32, kind="ExternalInput").ap()
    out = nc.dram_tensor("out", [G, D], F32, kind="ExternalOutput").ap()

    w1r = w1.rearrange("(do p) f -> p do f", p=P)
    w3r = w3.rearrange("(do p) f -> p do f", p=P)
    w2r = w2.rearrange("(fo p) d -> p fo d", p=P)

    with tile.TileContext(nc) as tc:
        dram = tc.alloc_tile_pool(name="dram", bufs=1, space="DRAM")
        # midT per g-half: mid_gh[h][p, fo, g'] = silu/up for f = fo*128+p,
        # g = h*1024+g'.  Phase-1 writes are 2KB/partition contiguous;
        # phase-2 loads are fo-chunked 8KB/partition contiguous.
        mid_gh = [dram.tile([P, FC, GH], BF16, tag=f"mid{h}", name=f"mid{h}") for h in range(2)]
        # w2 recast to bf16: w2b[p, fo, d] = w2[fo*128+p, d]
        w2b = dram.tile([P, FC, D], BF16, tag="w2b")

        # phase-2 w2 panels — allocated up front so the first panel can be
        # prefetched during the phase-1 tail
        w2p = tc.alloc_tile_pool(name="w2p", bufs=2, side="right")
        w2q_tiles = {}

        xtp = tc.alloc_tile_pool(name="xtp", bufs=1)
        xTs = [xtp.tile([P, DO, GH], BF16, tag=f"xT{h}", name=f"xT{h}") for h in range(2)]

        # ---- phase 0: x [G, D] fp32 -> xT [d_in, d_out, g] bf16
        p0 = tc.alloc_tile_pool(name="p0", bufs=4)
        idp = tc.alloc_tile_pool(name="idp", bufs=1)
        p0ps = tc.alloc_tile_pool(name="p0ps", bufs=6, space="PSUM")
        ident = idp.tile([P, P], BF16)
        make_identity(nc, ident)
        for go in range(GO):
            h, gl = (0, go) if go < 8 else (1, go - 8)
            for q in range(4):
                xsq = p0.tile([P, 512], F32, tag="xs")
                nc.sync.dma_start(
                    xsq, x[go * P : (go + 1) * P, q * 512 : (q + 1) * 512]
                )
                # pre-cast to bf16 on DVE so the PE transpose runs at
                # 1 cycle/row instead of fp32's 2
                xb = p0.tile([P, 512], BF16, tag="xb")
                nc.vector.tensor_copy(xb, xsq)
                tp = p0ps.tile([P, 4, P], BF16, tag="tp")
                for j in range(4):
                    nc.tensor.transpose(tp[:, j], xb[:, j * P : (j + 1) * P], ident)
                nc.scalar.copy(
                    xTs[h][:, q * 4 : (q + 1) * 4, gl * P : (gl + 1) * P], tp
                )
        idp.release()
        p0.release()
        p0ps.release()

        # ---- phase 1: midT[f, g] = silu(w1.T x) * (w3.T x), spill bf16
        wp = tc.alloc_tile_pool(name="wp", bufs=3)
        mp = tc.alloc_tile_pool(name="mp", bufs=3)
        wcp = tc.alloc_tile_pool(name="wcp", bufs=2)
        ps1g = tc.alloc_tile_pool(name="ps1g", bufs=2, space="PSUM")
        ps1u = tc.alloc_tile_pool(name="ps1u", bufs=2, space="PSUM")
        for fc in range(FC):
            w1t = wp.tile([P, DO, P], BF16, tag="w1")
            nc.gpsimd.dma_start(w1t, w1r[:, :, fc * P : (fc + 1) * P])
            w3t = wp.tile([P, DO, P], BF16, tag="w3")
            nc.gpsimd.dma_start(w3t, w3r[:, :, fc * P : (fc + 1) * P])
            # side stream: recast one f-row-block of w2 to bf16 in DRAM
            # (cast DMAs must go via gpsimd/SWDGE)
            w2c = wcp.tile([P, D], BF16, tag="w2c")
            nc.gpsimd.dma_start(w2c, w2r[:, fc, :])
            nc.sync.dma_start(w2b[:, fc, :], w2c)
            for h in range(2):
                pg = ps1g.tile([P, 2, 512], F32, tag="pg")
                pu = ps1u.tile([P, 2, 512], F32, tag="pu")
                for d in range(DO):
                    st, sp_ = (d == 0), (d == DO - 1)
                    for j in range(2):
                        nc.tensor.matmul(
                            pg[:, j],
                            w1t[:, d],
                            xTs[h][:, d, j * 512 : (j + 1) * 512],
                            start=st,
                            stop=sp_,
                        )
                    for j in range(2):
                        nc.tensor.matmul(
                            pu[:, j],
                            w3t[:, d],
                            xTs[h][:, d, j * 512 : (j + 1) * 512],
                            start=st,
                            stop=sp_,
                        )
                mo = mp.tile([P, 2 * 512], BF16, tag="mo")
                nc.scalar.activation(
                    mo, pg.rearrange("p j g -> p (j g)"),
                    mybir.ActivationFunctionType.Silu,
                )
                nc.vector.tensor_mul(mo, mo, pu.rearrange("p j g -> p (j g)"))
                nc.scalar.dma_start(mid_gh[h][:, fc, :], mo)
            if fc >= 40:
                # prefetch the first phase-2 w2 panel (h=0, dq=0) in four
                # fo-chunks; chunk k only needs w2b rows fc <= 11k+10,
                # already recast by this point in the stream
                k = fc - 40
                if k == 0:
                    w2q_tiles[0] = w2p.tile([P, FC, 512], BF16, tag="w2q", name="w2q")
                nc.gpsimd.dma_start(
                    w2q_tiles[0][:, 11 * k : 11 * (k + 1), :],
                    w2b[:, 11 * k : 11 * (k + 1), 0:512],
                )
        wcp.release()
        mp.release()
        wp.release()
        xtp.release()
        ps1u.release()
        ps1g.release()

        # ---- phase 2: out[g, d] = midT.T @ w2 (bf16 x bf16, fp32 psum)
        mh = tc.alloc_tile_pool(name="mh", bufs=1, side="right")
        op = tc.alloc_tile_pool(name="op", bufs=8, side="right")
        ps2 = tc.alloc_tile_pool(name="ps2", bufs=1, space="PSUM")
        for h in range(2):
            midH = mh.tile([P, FC, GH], BF16, tag="midH")
            # fo-chunked so compute chases the load; the first chunks are
            # smaller to cut the first-matmul latency at the phase boundary
            bounds = [0, 1, 2, 4, 8, 12, 16, 20, 24, 28, 32, 36, 40, FC]
            for c in range(len(bounds) - 1):
                lo, hi = bounds[c], bounds[c + 1]
                nc.sync.dma_start(
                    midH[:, lo:hi, :], mid_gh[h][:, lo:hi, :]
                )
            # dq order reversed on h=1 so the dq=3 panel (still in its pool
            # slot) is reused across the half boundary without a reload
            dqs = [0, 1, 2, 3] if h == 0 else [3, 2, 1, 0]
            for dq in dqs:
                if dq not in w2q_tiles:
                    w2q_tiles[dq] = w2p.tile([P, FC, 512], BF16, tag="w2q", name="w2q")
                    nc.gpsimd.dma_start(
                        w2q_tiles[dq], w2b[:, :, dq * 512 : (dq + 1) * 512]
                    )
                w2q = w2q_tiles[dq]
                po = [ps2.tile([P, 512], F32, tag=f"po{gp}", name=f"po{gp}") for gp in range(8)]
                for fo in range(FC):
                    st, sp_ = (fo == 0), (fo == FC - 1)
                    for gp in range(8):
                        nc.tensor.matmul(
                            po[gp],
                            midH[:, fo, gp * P : (gp + 1) * P],
                            w2q[:, fo],
                            start=st,
                            stop=sp_,
                        )
                        if sp_:
                            # drain each bank as soon as its group stops so
                            # the tail isn't serialized after the last MM;
                            # alternate engines/queues to halve the tail
                            ot = op.tile([P, 512], F32, tag="ot")
                            nc.vector.tensor_copy(ot, po[gp])
                            g0 = h * GH + gp * P
                            nc.scalar.dma_start(
                                out[g0 : g0 + P, dq * 512 : (dq + 1) * 512], ot
                            )
            # keep only the last-used panel live across the half boundary
            last = dqs[-1]
            w2q_tiles = {last: w2q_tiles[last]}
        op.release()
        mh.release()
        w2p.release()
        ps2.release()
        dram.release()
    nc.compile()
    return nc


_NC_CACHE = None


def _get_nc():
    global _NC_CACHE
    if _NC_CACHE is None:
        _NC_CACHE = build_nc()
    return _NC_CACHE


def _in_maps(routed_in_egD, w1, w2, w3):
    x = np.ascontiguousarray(np.asarray(routed_in_egD, dtype=np.float32))
    w1 = np.ascontiguousarray(np.asarray(w1, dtype=np.float32))
    w2 = np.ascontiguousarray(np.asarray(w2, dtype=np.float32))
    w3 = np.ascontiguousarray(np.asarray(w3, dtype=np.float32))
    x_e = x.reshape(E, G, D)
    return [
        {"x": x_e[e], "w1": w1[e], "w2": w2[e], "w3": w3[e]} for e in range(E)
    ]


def kernel(routed_in_egD, w1, w2, w3):
    nc = _get_nc()
    in_maps = _in_maps(routed_in_egD, w1, w2, w3)
    try:
        res = run_bass_kernel_spmd(nc, in_maps, core_ids=list(range(E)))
    except Exception:
        # the first execute after process start occasionally dies with a
        # transient NRT_EXEC_UNIT_UNRECOVERABLE through the PJRT tunnel;
        # a straight retry has always succeeded
        res = run_bass_kernel_spmd(nc, in_maps, core_ids=list(range(E)))
    return np.concatenate([r["out"] for r in res.results], axis=0)


def run_traced(routed_in_egD, w1, w2, w3, **trace_kwargs):
    """For test.py: run with NTFF tracing; returns (full_out, BassKernelResults)."""
    nc = _get_nc()
    res = run_bass_kernel_spmd(
        nc,
        _in_maps(routed_in_egD, w1, w2, w3),
        core_ids=list(range(E)),
        trace=True,
        **trace_kwargs,
    )
    out = np.concatenate([r["out"] for r in res.results], axis=0)
    return out, res


# revision 4
# speedup vs baseline: 1.0094x; 1.0094x over previous
"""Expert-parallel SwiGLU MLP (MoE experts) for 8 Trainium2 NeuronCores — v2.

Problem: routed_in_egD [E*G, D] fp32, w1/w3 [E, D, F], w2 [E, F, D], E=8,
G=2048, D=2048, F=5632.  reference:
    x_egD = routed.reshape(E, G, D)
    mid   = silu(x @ w1) * (x @ w3)          # [E, G, F]
    out   = (mid @ w2).reshape(E*G, D)

Sharding: expert-parallel — core e gets expert e's x slice + weights; no
collectives.  Each core runs three 2048x2048x5632-class GEMMs (~142 GFLOP).

Design (vs the fp32r baseline): all-bf16 matmul pipeline (same PE cycle
count as fp32r at 512 moving, half the SBUF footprint, no fp32r HW-rate
risk); phase-2 re-reads of mid eliminated (92MB -> 23MB) by keeping a
g-half of mid SBUF-resident and accumulating fo-outer into 8 PSUM banks;
w2 pre-cast to bf16 in DRAM spread across phase 1 so phase-2 HBM demand
drops 155->86MB; first w2 panel prefetched during the phase-1 tail; the
g-half mid panel is four 22KB tiles address-placed (pool order + pad)
over early-dead phase-1 tiles, and the final iteration pair is flipped,
so the phase-1->2 transition and the half-boundary reload are hidden.

Per-core kernel:
  phase 0: x [G,D] fp32 loaded, DVE-cast to bf16, PE-transpose ->
           xT0/xT1 [D, 1024] bf16 resident in SBUF.
  phase 1: per f-chunk (128 rows of F), per g-half: gateT/upT = w1/w3.T @ x
           accumulated over D in PSUM; SwiGLU (ACT silu -> bf16, DVE mul);
           midT [F,G] spilled to DRAM bf16 per g-half.  Side stream: w2
           fp32 -> bf16 recast via SBUF into DRAM scratch.
  phase 2: per g-half: midH [F,1024] bf16 SBUF-resident (fo-chunked load
           chases compute); per d-quarter: out[g,d] += midT.T @ w2, fo-outer
           over 8 PSUM banks (one per 128-g block).  dq order reversed on
           the second half so the last w2 panel is reused across halves;
           per-bank drains emitted at the stop matmul.
"""

import numpy as np

import concourse.mybir as mybir
import concourse.tile as tile
from concourse import bacc
from concourse.bass_utils import run_bass_kernel_spmd
from concourse.masks import make_identity

E, G, D, F = 8, 2048, 2048, 5632
P = 128
DO = D // P      # 16 d-chunks
FC = F // P      # 44 f-chunks
GO = G // P      # 16 g-chunks
GH = G // 2      # 1024 g-half

F32 = mybir.dt.float32
BF16 = mybir.dt.bfloat16


def build_nc():
    nc = bacc.Bacc("TRN2", target_bir_lowering=False)
    x = nc.dram_tensor("x", [G, D], F32, kind="ExternalInput").ap()
    w1 = nc.dram_tensor("w1", [D, F], F32, kind="ExternalInput").ap()
    w2 = nc.dram_tensor("w2", [F, D], F32, kind="ExternalInput").ap()
    w3 = nc.dram_tensor("w3", [D, F], F32, kind="ExternalInput").ap()
    out = nc.dram_tensor("out", [G, D], F32, kind="ExternalOutput").ap()

    w1r = w1.rearrange("(do p) f -> p do f", p=P)
    w3r = w3.rearrange("(do p) f -> p do f", p=P)
    w2r = w2.rearrange("(fo p) d -> p fo d", p=P)

    with tile.TileContext(nc) as tc:
        dram = tc.alloc_tile_pool(name="dram", bufs=1, space="DRAM")
        # midT per g-half: mid_gh[h][p, fo, g'] = silu/up for f = fo*128+p,
        # g = h*1024+g'.  Phase-1 writes are 2KB/partition contiguous;
        # phase-2 loads are fo-chunked 8KB/partition contiguous.
        mid_gh = [dram.tile([P, FC, GH], BF16, tag=f"mid{h}", name=f"mid{h}") for h in range(2)]
        # w2 recast to bf16: w2b[p, fo, d] = w2[fo*128+p, d]
        w2b = dram.tile([P, FC, D], BF16, tag="w2b")

        # phase-2 w2 panels — allocated up front so the first panel can be
        # prefetched during the phase-1 tail
        w2p = tc.alloc_tile_pool(name="w2p", bufs=2, side="right")
        w2q_tiles = {}

        # Left-side pool order places xT1 at the address range phase-2's
        # first mid-panel tile will reuse; combined with the flipped final
        # iteration pair (xT1's last reader one iteration early), the first
        # mid-panel load starts ~14us before phase 1 ends.
        wcp = tc.alloc_tile_pool(name="wcp", bufs=2)
        mp = tc.alloc_tile_pool(name="mp", bufs=3)
        xtp = tc.alloc_tile_pool(name="xtp", bufs=1)
        xT1 = xtp.tile([P, DO, GH], BF16, tag="xT1")
        xT0 = xtp.tile([P, DO, GH], BF16, tag="xT0")
        xTs = [xT0, xT1]

        # ---- phase 0: x [G, D] fp32 -> xT [d_in, d_out, g] bf16
        p0 = tc.alloc_tile_pool(name="p0", bufs=4)
        idp = tc.alloc_tile_pool(name="idp", bufs=1)
        p0ps = tc.alloc_tile_pool(name="p0ps", bufs=6, space="PSUM")
        ident = idp.tile([P, P], BF16)
        make_identity(nc, ident)
        for go in range(GO):
            h, gl = (0, go) if go < 8 else (1, go - 8)
            for q in range(4):
                xsq = p0.tile([P, 512], F32, tag="xs")
                nc.sync.dma_start(
                    xsq, x[go * P : (go + 1) * P, q * 512 : (q + 1) * 512]
                )
                # pre-cast to bf16 on DVE so the PE transpose runs at
                # 1 cycle/row instead of fp32's 2
                xb = p0.tile([P, 512], BF16, tag="xb")
                nc.vector.tensor_copy(xb, xsq)
                tp = p0ps.tile([P, 4, P], BF16, tag="tp")
                for j in range(4):
                    nc.tensor.transpose(tp[:, j], xb[:, j * P : (j + 1) * P], ident)
                nc.scalar.copy(
                    xTs[h][:, q * 4 : (q + 1) * 4, gl * P : (gl + 1) * P], tp
                )
        idp.release()
        p0.release()
        p0ps.release()

        # ---- phase 1: midT[f, g] = silu(w1.T x) * (w3.T x), spill bf16
        wp = tc.alloc_tile_pool(name="wp", bufs=3)
        ps1g = tc.alloc_tile_pool(name="ps1g", bufs=2, space="PSUM")
        ps1u = tc.alloc_tile_pool(name="ps1u", bufs=2, space="PSUM")
        w1ts, w3ts = {}, {}
        # final pair flipped: xT1's last reader is (43,1), one iteration
        # before the end, so phase-2 tiles placed over xT1 unblock early
        sched = []
        for f in range(FC):
            sched += [(f, 1), (f, 0)] if f == FC - 1 else [(f, 0), (f, 1)]
        for fc, h in sched:
            if fc not in w1ts:
                w1t = wp.tile([P, DO, P], BF16, tag="w1")
                nc.gpsimd.dma_start(w1t, w1r[:, :, fc * P : (fc + 1) * P])
                w3t = wp.tile([P, DO, P], BF16, tag="w3")
                nc.gpsimd.dma_start(w3t, w3r[:, :, fc * P : (fc + 1) * P])
                w1ts[fc], w3ts[fc] = w1t, w3t
                # side stream: recast one f-row-block of w2 to bf16 in DRAM
                # (cast DMAs must go via gpsimd/SWDGE)
                w2c = wcp.tile([P, D], BF16, tag="w2c")
                nc.gpsimd.dma_start(w2c, w2r[:, fc, :])
                nc.sync.dma_start(w2b[:, fc, :], w2c)
            w1t, w3t = w1ts[fc], w3ts[fc]
            if True:
                pg = ps1g.tile([P, 2, 512], F32, tag="pg")
                pu = ps1u.tile([P, 2, 512], F32, tag="pu")
                for d in range(DO):
                    st, sp_ = (d == 0), (d == DO - 1)
                    for j in range(2):
                        nc.tensor.matmul(
                            pg[:, j],
                            w1t[:, d],
                            xTs[h][:, d, j * 512 : (j + 1) * 512],
                            start=st,
                            stop=sp_,
                        )
                    for j in range(2):
                        nc.tensor.matmul(
                            pu[:, j],
                            w3t[:, d],
                            xTs[h][:, d, j * 512 : (j + 1) * 512],
                            start=st,
                            stop=sp_,
                        )
                mo = mp.tile([P, 2 * 512], BF16, tag="mo")
                nc.scalar.activation(
                    mo, pg.rearrange("p j g -> p (j g)"),
                    mybir.ActivationFunctionType.Silu,
                )
                nc.vector.tensor_mul(mo, mo, pu.rearrange("p j g -> p (j g)"))
                nc.scalar.dma_start(mid_gh[h][:, fc, :], mo)
            if fc >= 40 and h == 1:
                # prefetch the first phase-2 w2 panel (h=0, dq=0) in four
                # fo-chunks; chunk k only needs w2b rows fc <= 11k+10,
                # already recast by this point in the stream
                k = fc - 40
                if k == 0:
                    w2q_tiles[0] = w2p.tile([P, FC, 512], BF16, tag="w2q", name="w2q")
                nc.gpsimd.dma_start(
                    w2q_tiles[0][:, 11 * k : 11 * (k + 1), :],
                    w2b[:, 11 * k : 11 * (k + 1), 0:512],
                )
        wp.release()
        xtp.release()
        mp.release()
        wcp.release()
        ps1u.release()
        ps1g.release()

        # ---- phase 2: out[g, d] = midT.T @ w2 (bf16 x bf16, fp32 psum)
        # The g-half mid panel is four 22KB tiles stacked right-side so the
        # fo 0..10 tile lands over xT1's (early-dead) address range: its
        # load starts an iteration before phase 1 ends and the first
        # phase-2 matmul has no semaphore wait.  An 8KB pad keeps the
        # boundaries aligned to the xT1/xT0 split.
        padp = tc.alloc_tile_pool(name="padp", bufs=1, side="right")
        pad = padp.tile([P, 4096], BF16, tag="pad")
        mhs = [
            tc.alloc_tile_pool(name=f"mh{k}", bufs=1, side="right")
            for k in (3, 2, 1, 0)
        ]
        mhs.reverse()  # mhs[k] holds fo 11k..11k+10; mh0 at the lowest range
        op = tc.alloc_tile_pool(name="op", bufs=8, side="right")
        ps2 = tc.alloc_tile_pool(name="ps2", bufs=1, space="PSUM")
        for h in range(2):
            midH = [
                mhs[k].tile([P, 11, GH], BF16, tag="midH", name=f"midH{k}")
                for k in range(4)
            ]
            # fo-chunked so compute chases the load; the first chunks are
            # smaller to cut the first-matmul latency at the phase boundary
            bounds = [0, 1, 2, 4, 8, 11, 15, 19, 22, 26, 30, 33, 37, 40, FC]
            for c in range(len(bounds) - 1):
                lo, hi = bounds[c], bounds[c + 1]
                if lo // 11 == (hi - 1) // 11:
                    nc.sync.dma_start(
                        midH[lo // 11][:, lo % 11 : lo % 11 + (hi - lo), :],
                        mid_gh[h][:, lo:hi, :],
                    )
                else:  # straddles a tile boundary: split
                    m = ((hi - 1) // 11) * 11
                    nc.sync.dma_start(
                        midH[lo // 11][:, lo % 11 : 11, :], mid_gh[h][:, lo:m, :]
                    )
                    nc.sync.dma_start(
                        midH[m // 11][:, 0 : hi - m, :], mid_gh[h][:, m:hi, :]
                    )
            # dq order reversed on h=1 so the dq=3 panel (still in its pool
            # slot) is reused across the half boundary without a reload
            dqs = [0, 1, 2, 3] if h == 0 else [3, 2, 1, 0]
            for dq in dqs:
                if dq not in w2q_tiles:
                    w2q_tiles[dq] = w2p.tile([P, FC, 512], BF16, tag="w2q", name="w2q")
                    nc.gpsimd.dma_start(
                        w2q_tiles[dq], w2b[:, :, dq * 512 : (dq + 1) * 512]
                    )
                w2q = w2q_tiles[dq]
                po = [ps2.tile([P, 512], F32, tag=f"po{gp}", name=f"po{gp}") for gp in range(8)]
                for fo in range(FC):
                    st, sp_ = (fo == 0), (fo == FC - 1)
                    for gp in range(8):
                        nc.tensor.matmul(
                            po[gp],
                            midH[fo // 11][:, fo % 11, gp * P : (gp + 1) * P],
                            w2q[:, fo],
                            start=st,
                            stop=sp_,
                        )
                        if sp_:
                            # drain each bank as soon as its group stops so
                            # the tail isn't serialized after the last MM.
                            # For the very last block, split the out DMAs
                            # across both HWDGE queues (sync is idle then)
                            ot = op.tile([P, 512], F32, tag="ot")
                            nc.vector.tensor_copy(ot, po[gp])
                            g0 = h * GH + gp * P
                            last_blk = h == 1 and dq == dqs[-1]
                            dma_eng = (
                                nc.sync if (last_blk and gp % 2 == 0) else nc.scalar
                            )
                            dma_eng.dma_start(
                                out[g0 : g0 + P, dq * 512 : (dq + 1) * 512], ot
                            )
            # keep only the last-used panel live across the half boundary
            last = dqs[-1]
            w2q_tiles = {last: w2q_tiles[last]}
        op.release()
        for mhp in mhs:  # mh0 allocated last -> released first (LIFO)
            mhp.release()
        padp.release()
        w2p.release()
        ps2.release()
        dram.release()
    nc.compile()
    return nc


_NC_CACHE = None


def _get_nc():
    global _NC_CACHE
    if _NC_CACHE is None:
        _NC_CACHE = build_nc()
    return _NC_CACHE


def _in_maps(routed_in_egD, w1, w2, w3):
    x = np.ascontiguousarray(np.asarray(routed_in_egD, dtype=np.float32))
    w1 = np.ascontiguousarray(np.asarray(w1, dtype=np.float32))
    w2 = np.ascontiguousarray(np.asarray(w2, dtype=np.float32))
    w3 = np.ascontiguousarray(np.asarray(w3, dtype=np.float32))
    x_e = x.reshape(E, G, D)
    return [
        {"x": x_e[e], "w1": w1[e], "w2": w2[e], "w3": w3[e]} for e in range(E)
    ]


def kernel(routed_in_egD, w1, w2, w3):
    nc = _get_nc()
    in_maps = _in_maps(routed_in_egD, w1, w2, w3)
    try:
        res = run_bass_kernel_spmd(nc, in_maps, core_ids=list(range(E)))
    except Exception:
        # the first execute after process start occasionally dies with a
        # transient NRT_EXEC_UNIT_UNRECOVERABLE through the PJRT tunnel;
        # a straight retry has always succeeded
        res = run_bass_kernel_spmd(nc, in_maps, core_ids=list(range(E)))
    return np.concatenate([r["out"] for r in res.results], axis=0)


def run_traced(routed_in_egD, w1, w2, w3, **trace_kwargs):
    """For test.py: run with NTFF tracing; returns (full_out, BassKernelResults)."""
    nc = _get_nc()
    res = run_bass_kernel_spmd(
        nc,
        _in_maps(routed_in_egD, w1, w2, w3),
        core_ids=list(range(E)),
        trace=True,
        **trace_kwargs,
    )
    out = np.concatenate([r["out"] for r in res.results], axis=0)
    return out, res
